# revision 12
# baseline (speedup 1.0000x reference)
"""MoE (top-4 of 16 experts, SwiGLU FFN) on 8 Trainium2 NeuronCores.

Strategy: expert parallelism. The router (x @ Wr, softmax, top-4) is 0.26% of
the FLOPs and runs on host; tokens are gathered per expert on host (the
"all-to-all dispatch"), each core runs the dense SwiGLU FFN for its 2 experts
on its gathered tokens in bf16 (fp32 PSUM accumulation), and the host
scatter-adds the weighted expert outputs back ("combine").

Shapes (hardcoded): B=4, S=1024, D=1024, E=16, F=512, TOPK=4. N = B*S = 4096.
Per core: 2 expert slots with static token capacities (C0, C1) (multiples of
128, data dependent). Each core puts its larger expert in slot 0.

All DRAM arrays are pre-tiled on host so every DMA is partition-contiguous
(128 descriptors of >=1 KiB instead of thousands of tiny ones).
"""

import numpy as np
import ml_dtypes

import concourse.bass as bass
import concourse.bacc as bacc
import concourse.tile as tile
from concourse import bass_utils, mybir

B, S, D = 4, 1024, 1024
E, F, TOPK = 16, 512, 4
N = B * S
NCORES = 8
EPC = E // NCORES  # experts per core
P = 128
DT = D // P  # 8
FT = F // P  # 4
TCH = 512    # token chunk (matmul moving free dim)

BF16 = ml_dtypes.bfloat16

_program_cache: dict[tuple, object] = {}


# ---------------------------------------------------------------- host router
def _route(xf: np.ndarray, Wr: np.ndarray):
    """Top-4 expert ids + renormalized weights per token.

    Renormalized top-k softmax weights == softmax over just the top-k logits,
    so the full softmax denominator is never needed.
    """
    logits = xf @ Wr  # [N, E] fp32
    idx = np.argpartition(-logits, TOPK - 1, axis=1)[:, :TOPK]  # [N, K]
    lt = np.take_along_axis(logits, idx, axis=1)
    lt = lt - lt.max(axis=1, keepdims=True)
    ex = np.exp(lt)
    w = ex / ex.sum(axis=1, keepdims=True)
    return idx, w.astype(np.float32)


def _chunks_of(C, lead=0):
    """Split C into chunks of <= 512 tokens, as even as possible.

    Every chunk size is a multiple of 128 except the last, which carries the
    remainder — chunk starts must sit on the global 128-token grid (stage-B
    m-groups and cw columns are partition-aligned). Even splits avoid tiny
    trailing chunks whose matmuls are LDWEIGHTS-bound. A `lead` chunk (e.g.
    256 tokens) lets the PE start before the full token block has arrived.
    """
    sizes = []
    if lead and C >= lead + P:
        sizes.append(lead)
        C -= lead
    n = -(-C // TCH)
    blocks = C // P            # whole 128-token blocks
    rem = C - blocks * P       # leftover tokens (< 128), go to the last chunk
    per = blocks // n
    extra = blocks - per * n   # first `extra` chunks get one more block
    sizes += [(per + 1) * P] * extra + [per * P] * (n - extra)
    sizes[-1] += rem
    out, t0 = [], 0
    for sz in sizes:
        out.append((t0, sz))
        t0 += sz
    return out


# ---------------------------------------------------------------- device code
def _build_program(caps: tuple):
    """One SPMD program: EPC expert slots with capacities caps[s].

    Inputs (per core), all pre-tiled partition-major on host:
      xt [NCHTOT, 128, DT, TCH] bf16   gathered+transposed tokens, per chunk
      wg [EPC, 128, DT, F]      bf16   wg[s, p, d, f] = Wg_slot_s[d*128+p, f]
      wu [EPC, 128, DT, F]      bf16
      wd [EPC, 128, FT, D]      bf16   wd[s, p, t, d] = Wd_slot_s[t*128+p, d]
      cw [CTOT//128, 128]       f32    combine weight per gathered token
    Output:
      y  [CTOT//128, 128, D]    bf16   cw * (silu(x@wg) * (x@wu)) @ wd
    """
    def r128(v):
        return max(P, -(-v // P) * P)

    lcaps = [r128(C) for C in caps]   # layout capacities (128-aligned)
    CTOT = sum(lcaps)
    slot_chunks = [_chunks_of(C, lead=256 if s == 0 else 0) for s, C in enumerate(caps)]
    # flat xt layout: chunks packed back to back, each [P, DT, tch] row-major
    xt_offs, xoff = [], 0
    for chs in slot_chunks:
        offs = []
        for (_, tch) in chs:
            offs.append(xoff)
            xoff += P * DT * tch
        xt_offs.append(offs)
    XTELEMS = xoff

    nc = bacc.Bacc("TRN2", target_bir_lowering=False, debug=False)
    bf = mybir.dt.bfloat16
    f32 = mybir.dt.float32

    xt = nc.declare_dram_parameter("xt", [XTELEMS], bf, isOutput=False)
    wg = nc.declare_dram_parameter("wg", [EPC, FT, P, DT, P], bf, isOutput=False)
    wu = nc.declare_dram_parameter("wu", [EPC, FT, P, DT, P], bf, isOutput=False)
    wd = nc.declare_dram_parameter("wd", [EPC, P, FT, D], bf, isOutput=False)
    cw = nc.declare_dram_parameter("cw", [P, CTOT // P], f32, isOutput=False)
    y = nc.declare_dram_parameter("y", [CTOT // P, P, D], bf, isOutput=True)

    with tile.TileContext(nc) as tc:
        with (
            tc.tile_pool(name="wpool", bufs=2) as wpool,
            tc.tile_pool(name="xpool", bufs=3) as xpool,
            tc.tile_pool(name="hpool", bufs=2) as hpool,
            tc.tile_pool(name="spool", bufs=3) as spool,
            tc.tile_pool(name="ypool", bufs=4) as ypool,
            tc.tile_pool(name="psA", bufs=2, space="PSUM") as psA,
            tc.tile_pool(name="psB", bufs=3, space="PSUM") as psB,
        ):
            ch_base = 0   # global chunk index (xt rows)
            off = 0       # global token offset (cw / y rows)
            for s in range(EPC):
                Cs = caps[s]
                chunks = slot_chunks[s]

                # DMA instruction issue costs ~640 ns of sequencer time each,
                # so startup uses few, large DMAs spread over the three
                # DMA-capable sequencers. Later experts' weights issue from
                # the scalar sequencer, whose ACT stream naturally delays them
                # past expert-0's startup (no bandwidth competition).
                NCC = -(-Cs // P)

                def xt_load(ci, tch, eng=None):
                    # contiguous SBUF destination (any partial-width dst
                    # slice explodes into 1024 descriptors, ~10us of
                    # descriptor generation); matmuls use a strided view
                    t = xpool.tile([P, DT * tch], bf, tag="xt")
                    src = xt[
                        xt_offs[s][ci] : xt_offs[s][ci] + P * DT * tch
                    ].rearrange("(p x) -> p x", p=P)
                    (eng or nc.sync).dma_start(t[:], src)
                    return t.rearrange("p (dt c) -> p dt c", dt=DT)

                xt0 = xt_load(0, chunks[0][1])
                wg_sb = wpool.tile([P, FT, DT, P], bf, tag="wg")
                wu_sb = wpool.tile([P, FT, DT, P], bf, tag="wu")
                cw_sb = wpool.tile([P, NCC], f32, tag="cw")
                wd_sb = wpool.tile([P, FT, D], bf, tag="wd")
                if s == 0:
                    # F-quarters: the first G matmul group needs only wg[f0]
                    for fq in range(FT):
                        nc.gpsimd.dma_start(wg_sb[:, fq], wg[s, fq])
                        nc.scalar.dma_start(wu_sb[:, fq], wu[s, fq])
                    nc.gpsimd.dma_start(
                        cw_sb[:], cw[:, off // P : off // P + NCC]
                    )
                    nc.gpsimd.dma_start(wd_sb[:], wd[s])
                else:
                    for fq in range(FT):
                        nc.scalar.dma_start(wg_sb[:, fq], wg[s, fq])
                        nc.scalar.dma_start(wu_sb[:, fq], wu[s, fq])
                    nc.scalar.dma_start(
                        cw_sb[:], cw[:, off // P : off // P + NCC]
                    )
                    nc.scalar.dma_start(wd_sb[:], wd[s])

                xt_tiles = {0: xt0}
                for ci, (t0, tch) in enumerate(chunks):
                    xt_sb = xt_tiles.pop(ci)
                    # stage A: h^T[f] = silu(G^T) * U^T, [F-part, tok-free].
                    # All G matmuls first (need only wg), then all U matmuls:
                    # chunk 0 can start before wu has arrived.
                    h_sb = hpool.tile([P, FT, TCH], bf, tag="h")
                    sgs = []
                    for f in range(FT):
                        pg = psA.tile([P, TCH], f32, tag="pg")
                        for d in range(DT):
                            nc.tensor.matmul(
                                pg[:, :tch],
                                lhsT=wg_sb[:, f, d, :],
                                rhs=xt_sb[:, d, :tch],
                                start=(d == 0),
                                stop=(d == DT - 1),
                            )
                        sg = spool.tile([P, TCH], f32, tag=f"sg{f}")
                        nc.scalar.activation(
                            sg[:, :tch],
                            pg[:, :tch],
                            mybir.ActivationFunctionType.Silu,
                        )
                        sgs.append(sg)
                        if f == 1 and ci + 1 < len(chunks):
                            # issue on scalar AFTER this silu in program
                            # order: keeps its traffic out of the startup
                            # bandwidth crunch
                            xt_tiles[ci + 1] = xt_load(
                                ci + 1, chunks[ci + 1][1], eng=nc.scalar
                            )
                    for f in range(FT):
                        pu = psA.tile([P, TCH], f32, tag="pu")
                        for d in range(DT):
                            nc.tensor.matmul(
                                pu[:, :tch],
                                lhsT=wu_sb[:, f, d, :],
                                rhs=xt_sb[:, d, :tch],
                                start=(d == 0),
                                stop=(d == DT - 1),
                            )
                        nc.vector.tensor_mul(
                            out=h_sb[:, f, :tch],
                            in0=sgs[f][:, :tch],
                            in1=pu[:, :tch],
                        )
                    # stage B: y[m] = cw * (h^T)^T @ wd, [tok-part, D-free]
                    for m in range(-(-tch // P)):
                        cc = (off + t0) // P + m
                        mr = min(P, tch - m * P)  # partial last m-group
                        y_sb = ypool.tile([P, D], bf, tag="y")
                        for dd in range(D // TCH):
                            py = psB.tile([P, TCH], f32, tag="py")
                            for f in range(FT):
                                nc.tensor.matmul(
                                    py[:mr],
                                    lhsT=h_sb[:, f, m * P : m * P + mr],
                                    rhs=wd_sb[:, f, dd * TCH : (dd + 1) * TCH],
                                    start=(f == 0),
                                    stop=(f == FT - 1),
                                )
                            nc.scalar.activation(
                                y_sb[:mr, dd * TCH : (dd + 1) * TCH],
                                py[:mr],
                                mybir.ActivationFunctionType.Copy,
                                scale=cw_sb[:mr, (cc - off // P) : (cc - off // P) + 1],
                            )
                        nc.gpsimd.dma_start(y[cc, :mr], y_sb[:mr])
                ch_base += len(chunks)
                off += lcaps[s]
    nc.compile()
    return nc


def _get_program(caps):
    if caps not in _program_cache:
        _program_cache[caps] = _build_program(caps)
    return _program_cache[caps]


# ------------------------------------------------------------------ profiling
def _ensure_ntff_hook():
    """The container's `antenv` stub lacks `axon_hooks`, so trn_boot's NTFF
    profile hook never gets registered and trace=True degrades to no-op.
    Register the module + ctypes hook at runtime."""
    import sys
    import types

    import antenv

    if "antenv.axon_hooks" not in sys.modules:
        mod = types.ModuleType("antenv.axon_hooks")
        mod._hook = None

        def set_axon_ntff_profile_hook(h):
            mod._hook = h

        def get_axon_ntff_profile_hook():
            return mod._hook

        mod.set_axon_ntff_profile_hook = set_axon_ntff_profile_hook
        mod.get_axon_ntff_profile_hook = get_axon_ntff_profile_hook
        sys.modules["antenv.axon_hooks"] = mod
        antenv.axon_hooks = mod
    mod = sys.modules["antenv.axon_hooks"]
    if mod._hook is None:
        from trn_agent_boot.trn_boot import _ntff_profile_via_ctypes

        mod.set_axon_ntff_profile_hook(
            _ntff_profile_via_ctypes("/opt/axon/libaxon_pjrt.so")
        )


# ---------------------------------------------------------------- entry point
def _run(inputs: dict, trace: bool = False):
    x = np.asarray(inputs["x"], dtype=np.float32)
    Wr = np.asarray(inputs["Wr"], dtype=np.float32)
    Wg = np.asarray(inputs["Wg"], dtype=np.float32)
    Wu = np.asarray(inputs["Wu"], dtype=np.float32)
    Wd = np.asarray(inputs["Wd"], dtype=np.float32)

    xf = x.reshape(N, D)
    idx, w = _route(xf, Wr)

    # group (token, weight) by expert
    flat_e = idx.ravel()
    flat_t = np.repeat(np.arange(N, dtype=np.int64), TOPK)
    flat_w = w.ravel()
    order = np.argsort(flat_e, kind="stable")
    ge, gt, gw = flat_e[order], flat_t[order], flat_w[order]
    counts = np.bincount(ge, minlength=E)
    starts = np.zeros(E + 1, dtype=np.int64)
    np.cumsum(counts, out=starts[1:])

    # global pairing: sort experts by count desc, core c gets ranks (c, 15-c);
    # slot 0 holds the larger one. Minimizes both slot capacities:
    # caps = (count of rank 0, count of rank NCORES) rounded up to 128.
    def r128(v):
        return max(P, int(-(-v // P)) * P)

    by_size = sorted(range(E), key=lambda e: -counts[e])
    slot_experts = [
        [by_size[c], by_size[E - 1 - c]] for c in range(NCORES)
    ]  # [core][slot] -> expert id
    caps = tuple(
        int(max(counts[slot_experts[c][s]] for c in range(NCORES)))
        for s in range(EPC)
    )
    lcaps = [r128(Cs) for Cs in caps]
    CTOT = sum(lcaps)
    slot_chunks = [
        _chunks_of(Cs, lead=256 if s == 0 else 0) for s, Cs in enumerate(caps)
    ]
    slot_off = np.cumsum([0] + list(lcaps))
    # flat xt layout: chunks packed back to back, each [P, DT, tch] row-major
    xt_offs, xoff = [], 0
    for chs in slot_chunks:
        offs = []
        for (_, tch) in chs:
            offs.append(xoff)
            xoff += P * DT * tch
        xt_offs.append(offs)
    XTELEMS = xoff

    xt_all = np.zeros((NCORES, XTELEMS), dtype=BF16)
    cw_all = np.zeros((NCORES, P, CTOT // P), dtype=np.float32)
    wg_all = np.zeros((NCORES, EPC, FT, P, DT, P), dtype=BF16)
    wu_all = np.zeros((NCORES, EPC, FT, P, DT, P), dtype=BF16)
    wd_all = np.zeros((NCORES, EPC, P, FT, D), dtype=BF16)

    tok_lists = {}
    for c in range(NCORES):
        for s in range(EPC):
            e = slot_experts[c][s]
            toks = gt[starts[e] : starts[e + 1]]
            tok_lists[(c, s)] = toks
            ne = len(toks)
            # tokens, transposed + tiled per chunk: [p, d, c] = X[tok, d*128+p]
            for ci, (t0, tch) in enumerate(slot_chunks[s]):
                sel = toks[t0 : min(t0 + tch, ne)]
                blk = np.zeros((P, DT, tch), dtype=BF16)
                if len(sel):
                    blk[:, :, : len(sel)] = (
                        xf[sel]
                        .astype(BF16)
                        .reshape(len(sel), DT, P)
                        .transpose(2, 1, 0)
                    )
                xo = xt_offs[s][ci]
                xt_all[c, xo : xo + P * DT * tch] = blk.ravel()
            cw_flat = np.zeros(lcaps[s], dtype=np.float32)
            cw_flat[:ne] = gw[starts[e] : starts[e + 1]]
            cw_all[c, :, slot_off[s] // P : slot_off[s + 1] // P] = (
                cw_flat.reshape(-1, P).T
            )
            # weights, partition-major K tiles
            wg_all[c, s] = (
                Wg[e].astype(BF16).reshape(DT, P, FT, P).transpose(2, 1, 0, 3)
            )
            wu_all[c, s] = (
                Wu[e].astype(BF16).reshape(DT, P, FT, P).transpose(2, 1, 0, 3)
            )
            wd_all[c, s] = Wd[e].astype(BF16).reshape(FT, P, D).transpose(1, 0, 2)

    nc = _get_program(caps)
    in_maps = [
        {
            "xt": xt_all[c],
            "wg": wg_all[c],
            "wu": wu_all[c],
            "wd": wd_all[c],
            "cw": cw_all[c],
        }
        for c in range(NCORES)
    ]
    kwargs = {}
    if trace:
        _ensure_ntff_hook()
        kwargs = dict(trace=True, trace_cores=list(range(NCORES)))
    res = bass_utils.run_bass_kernel_spmd(
        nc, in_maps, core_ids=list(range(NCORES)), **kwargs
    )

    out = np.zeros((N, D), dtype=np.float32)
    for c in range(NCORES):
        yc = res.results[c]["y"].reshape(CTOT, D)
        for s in range(EPC):
            toks = tok_lists[(c, s)]
            out[toks] += yc[slot_off[s] : slot_off[s] + len(toks)].astype(
                np.float32
            )
    return out.reshape(B, S, D), res.exec_time_ns


# Pre-register the NTFF hook shim at import: if the grading harness sets
# BASS_TRACE=1, run_bass_kernel_spmd's axon trace path imports
# antenv.axon_hooks, which the container's antenv stub lacks.
try:
    _ensure_ntff_hook()
except Exception:
    pass


def kernel(**inputs) -> np.ndarray:
    out, _ = _run(inputs, trace=False)
    return out


# revision 13
# speedup vs baseline: 1.0078x; 1.0078x over previous
"""MoE (top-4 of 16 experts, SwiGLU FFN) on 8 Trainium2 NeuronCores.

Strategy: expert parallelism. The router (x @ Wr, softmax, top-4) is 0.26% of
the FLOPs and runs on host; tokens are gathered per expert on host (the
"all-to-all dispatch"), each core runs the dense SwiGLU FFN for its 2 experts
on its gathered tokens in bf16 (fp32 PSUM accumulation), and the host
scatter-adds the weighted expert outputs back ("combine").

Shapes (hardcoded): B=4, S=1024, D=1024, E=16, F=512, TOPK=4. N = B*S = 4096.
Per core: 2 expert slots with static token capacities (C0, C1) (multiples of
128, data dependent). Each core puts its larger expert in slot 0.

All DRAM arrays are pre-tiled on host so every DMA is partition-contiguous
(128 descriptors of >=1 KiB instead of thousands of tiny ones).
"""

import numpy as np
import ml_dtypes

import concourse.bass as bass
import concourse.bacc as bacc
import concourse.tile as tile
from concourse import bass_utils, mybir

B, S, D = 4, 1024, 1024
E, F, TOPK = 16, 512, 4
N = B * S
NCORES = 8
EPC = E // NCORES  # experts per core
P = 128
DT = D // P  # 8
FT = F // P  # 4
TCH = 512    # token chunk (matmul moving free dim)

BF16 = ml_dtypes.bfloat16

_program_cache: dict[tuple, object] = {}


# ---------------------------------------------------------------- host router
def _route(xf: np.ndarray, Wr: np.ndarray):
    """Top-4 expert ids + renormalized weights per token.

    Renormalized top-k softmax weights == softmax over just the top-k logits,
    so the full softmax denominator is never needed.
    """
    logits = xf @ Wr  # [N, E] fp32
    idx = np.argpartition(-logits, TOPK - 1, axis=1)[:, :TOPK]  # [N, K]
    lt = np.take_along_axis(logits, idx, axis=1)
    lt = lt - lt.max(axis=1, keepdims=True)
    ex = np.exp(lt)
    w = ex / ex.sum(axis=1, keepdims=True)
    return idx, w.astype(np.float32)


def _chunks_of(C, lead=0):
    """Split C into chunks of <= 512 tokens, as even as possible.

    Every chunk size is a multiple of 128 except the last, which carries the
    remainder — chunk starts must sit on the global 128-token grid (stage-B
    m-groups and cw columns are partition-aligned). Even splits avoid tiny
    trailing chunks whose matmuls are LDWEIGHTS-bound. A `lead` chunk (e.g.
    256 tokens) lets the PE start before the full token block has arrived.
    """
    sizes = []
    if lead and C >= lead + P:
        sizes.append(lead)
        C -= lead
    n = -(-C // TCH)
    blocks = C // P            # whole 128-token blocks
    rem = C - blocks * P       # leftover tokens (< 128), go to the last chunk
    per = blocks // n
    extra = blocks - per * n   # first `extra` chunks get one more block
    sizes += [(per + 1) * P] * extra + [per * P] * (n - extra)
    sizes[-1] += rem
    out, t0 = [], 0
    for sz in sizes:
        out.append((t0, sz))
        t0 += sz
    return out


# ---------------------------------------------------------------- device code
def _build_program(caps: tuple):
    """One SPMD program: EPC expert slots with capacities caps[s].

    Inputs (per core), all pre-tiled partition-major on host:
      xt [NCHTOT, 128, DT, TCH] bf16   gathered+transposed tokens, per chunk
      wg [EPC, 128, DT, F]      bf16   wg[s, p, d, f] = Wg_slot_s[d*128+p, f]
      wu [EPC, 128, DT, F]      bf16
      wd [EPC, 128, FT, D]      bf16   wd[s, p, t, d] = Wd_slot_s[t*128+p, d]
      cw [CTOT//128, 128]       f32    combine weight per gathered token
    Output:
      y  [CTOT//128, 128, D]    bf16   cw * (silu(x@wg) * (x@wu)) @ wd
    """
    def r128(v):
        return max(P, -(-v // P) * P)

    lcaps = [r128(C) for C in caps]   # layout capacities (128-aligned)
    CTOT = sum(lcaps)
    slot_chunks = [_chunks_of(C, lead=256 if s == 0 else 0) for s, C in enumerate(caps)]
    # flat xt layout: chunks packed back to back, each [P, DT, tch] row-major
    xt_offs, xoff = [], 0
    for chs in slot_chunks:
        offs = []
        for (_, tch) in chs:
            offs.append(xoff)
            xoff += P * DT * tch
        xt_offs.append(offs)
    XTELEMS = xoff

    nc = bacc.Bacc("TRN2", target_bir_lowering=False, debug=False)
    bf = mybir.dt.bfloat16
    f32 = mybir.dt.float32

    xt = nc.declare_dram_parameter("xt", [XTELEMS], bf, isOutput=False)
    wg = nc.declare_dram_parameter("wg", [EPC, FT, P, DT, P], bf, isOutput=False)
    wu = nc.declare_dram_parameter("wu", [EPC, FT, P, DT, P], bf, isOutput=False)
    wd = nc.declare_dram_parameter("wd", [EPC, P, FT, D], bf, isOutput=False)
    cw = nc.declare_dram_parameter("cw", [P, CTOT // P], f32, isOutput=False)
    y = nc.declare_dram_parameter("y", [CTOT // P, P, D], bf, isOutput=True)

    with tile.TileContext(nc) as tc:
        with (
            tc.tile_pool(name="wpool", bufs=2) as wpool,
            tc.tile_pool(name="xpool", bufs=3) as xpool,
            tc.tile_pool(name="hpool", bufs=2) as hpool,
            tc.tile_pool(name="spool", bufs=3) as spool,
            tc.tile_pool(name="ypool", bufs=4) as ypool,
            tc.tile_pool(name="psA", bufs=2, space="PSUM") as psA,
            tc.tile_pool(name="psB", bufs=3, space="PSUM") as psB,
        ):
            ch_base = 0   # global chunk index (xt rows)
            off = 0       # global token offset (cw / y rows)
            for s in range(EPC):
                Cs = caps[s]
                chunks = slot_chunks[s]

                # DMA instruction issue costs ~640 ns of sequencer time each,
                # so startup uses few, large DMAs spread over the three
                # DMA-capable sequencers. Later experts' weights issue from
                # the scalar sequencer, whose ACT stream naturally delays them
                # past expert-0's startup (no bandwidth competition).
                NCC = -(-Cs // P)

                def xt_load(ci, tch, eng=None):
                    # contiguous SBUF destination (any partial-width dst
                    # slice explodes into 1024 descriptors, ~10us of
                    # descriptor generation); matmuls use a strided view
                    t = xpool.tile([P, DT * tch], bf, tag="xt")
                    src = xt[
                        xt_offs[s][ci] : xt_offs[s][ci] + P * DT * tch
                    ].rearrange("(p x) -> p x", p=P)
                    (eng or nc.sync).dma_start(t[:], src)
                    return t.rearrange("p (dt c) -> p dt c", dt=DT)

                # slot-1's first chunk loads via scalar so it never
                # queues behind the slot-0 y writebacks on sync
                xt0 = xt_load(0, chunks[0][1], eng=nc.sync if s == 0 else nc.scalar)
                wg_sb = wpool.tile([P, FT, DT, P], bf, tag="wg")
                wu_sb = wpool.tile([P, FT, DT, P], bf, tag="wu")
                cw_sb = wpool.tile([P, NCC], f32, tag="cw")
                wd_sb = wpool.tile([P, FT, D], bf, tag="wd")
                if s == 0:
                    # F-quarters: the first G matmul group needs only wg[f0]
                    for fq in range(FT):
                        nc.gpsimd.dma_start(wg_sb[:, fq], wg[s, fq])
                        nc.scalar.dma_start(wu_sb[:, fq], wu[s, fq])
                    nc.gpsimd.dma_start(
                        cw_sb[:], cw[:, off // P : off // P + NCC]
                    )
                    nc.gpsimd.dma_start(wd_sb[:], wd[s])
                else:
                    for fq in range(FT):
                        nc.scalar.dma_start(wg_sb[:, fq], wg[s, fq])
                        nc.scalar.dma_start(wu_sb[:, fq], wu[s, fq])
                    nc.scalar.dma_start(
                        cw_sb[:], cw[:, off // P : off // P + NCC]
                    )
                    nc.scalar.dma_start(wd_sb[:], wd[s])

                xt_tiles = {0: xt0}
                for ci, (t0, tch) in enumerate(chunks):
                    xt_sb = xt_tiles.pop(ci)
                    # stage A: h^T[f] = silu(G^T) * U^T, [F-part, tok-free].
                    # All G matmuls first (need only wg), then all U matmuls:
                    # chunk 0 can start before wu has arrived.
                    h_sb = hpool.tile([P, FT, TCH], bf, tag="h")
                    sgs = []
                    for f in range(FT):
                        pg = psA.tile([P, TCH], f32, tag="pg")
                        for d in range(DT):
                            nc.tensor.matmul(
                                pg[:, :tch],
                                lhsT=wg_sb[:, f, d, :],
                                rhs=xt_sb[:, d, :tch],
                                start=(d == 0),
                                stop=(d == DT - 1),
                            )
                        sg = spool.tile([P, TCH], f32, tag=f"sg{f}")
                        nc.scalar.activation(
                            sg[:, :tch],
                            pg[:, :tch],
                            mybir.ActivationFunctionType.Silu,
                        )
                        sgs.append(sg)
                        if f == 1 and ci + 1 < len(chunks):
                            # issue on scalar AFTER this silu in program
                            # order: keeps its traffic out of the startup
                            # bandwidth crunch
                            xt_tiles[ci + 1] = xt_load(
                                ci + 1, chunks[ci + 1][1], eng=nc.scalar
                            )
                    for f in range(FT):
                        pu = psA.tile([P, TCH], f32, tag="pu")
                        for d in range(DT):
                            nc.tensor.matmul(
                                pu[:, :tch],
                                lhsT=wu_sb[:, f, d, :],
                                rhs=xt_sb[:, d, :tch],
                                start=(d == 0),
                                stop=(d == DT - 1),
                            )
                        nc.vector.tensor_mul(
                            out=h_sb[:, f, :tch],
                            in0=sgs[f][:, :tch],
                            in1=pu[:, :tch],
                        )
                    # stage B: y[m] = cw * (h^T)^T @ wd, [tok-part, D-free]
                    for m in range(-(-tch // P)):
                        cc = (off + t0) // P + m
                        mr = min(P, tch - m * P)  # partial last m-group
                        y_sb = ypool.tile([P, D], bf, tag="y")
                        for dd in range(D // TCH):
                            py = psB.tile([P, TCH], f32, tag="py")
                            for f in range(FT):
                                nc.tensor.matmul(
                                    py[:mr],
                                    lhsT=h_sb[:, f, m * P : m * P + mr],
                                    rhs=wd_sb[:, f, dd * TCH : (dd + 1) * TCH],
                                    start=(f == 0),
                                    stop=(f == FT - 1),
                                )
                            nc.scalar.activation(
                                y_sb[:mr, dd * TCH : (dd + 1) * TCH],
                                py[:mr],
                                mybir.ActivationFunctionType.Copy,
                                scale=cw_sb[:mr, (cc - off // P) : (cc - off // P) + 1],
                            )
                        # y writebacks on sync (hardware DGE): gpsimd DMAs
                        # go through the software DGE ring whose teardown
                        # drain costs ~85ns per queued entry (measured);
                        # keeping gpsimd to 6 entries saves ~1.5us of epilogue
                        nc.sync.dma_start(y[cc, :mr], y_sb[:mr])
                ch_base += len(chunks)
                off += lcaps[s]
    nc.compile()
    return nc


def _get_program(caps):
    if caps not in _program_cache:
        _program_cache[caps] = _build_program(caps)
    return _program_cache[caps]


# ------------------------------------------------------------------ profiling
def _ensure_ntff_hook():
    """The container's `antenv` stub lacks `axon_hooks`, so trn_boot's NTFF
    profile hook never gets registered and trace=True degrades to no-op.
    Register the module + ctypes hook at runtime."""
    import sys
    import types

    import antenv

    if "antenv.axon_hooks" not in sys.modules:
        mod = types.ModuleType("antenv.axon_hooks")
        mod._hook = None

        def set_axon_ntff_profile_hook(h):
            mod._hook = h

        def get_axon_ntff_profile_hook():
            return mod._hook

        mod.set_axon_ntff_profile_hook = set_axon_ntff_profile_hook
        mod.get_axon_ntff_profile_hook = get_axon_ntff_profile_hook
        sys.modules["antenv.axon_hooks"] = mod
        antenv.axon_hooks = mod
    mod = sys.modules["antenv.axon_hooks"]
    if mod._hook is None:
        from trn_agent_boot.trn_boot import _ntff_profile_via_ctypes

        mod.set_axon_ntff_profile_hook(
            _ntff_profile_via_ctypes("/opt/axon/libaxon_pjrt.so")
        )


# ---------------------------------------------------------------- entry point
def _run(inputs: dict, trace: bool = False):
    x = np.asarray(inputs["x"], dtype=np.float32)
    Wr = np.asarray(inputs["Wr"], dtype=np.float32)
    Wg = np.asarray(inputs["Wg"], dtype=np.float32)
    Wu = np.asarray(inputs["Wu"], dtype=np.float32)
    Wd = np.asarray(inputs["Wd"], dtype=np.float32)

    xf = x.reshape(N, D)
    idx, w = _route(xf, Wr)

    # group (token, weight) by expert
    flat_e = idx.ravel()
    flat_t = np.repeat(np.arange(N, dtype=np.int64), TOPK)
    flat_w = w.ravel()
    order = np.argsort(flat_e, kind="stable")
    ge, gt, gw = flat_e[order], flat_t[order], flat_w[order]
    counts = np.bincount(ge, minlength=E)
    starts = np.zeros(E + 1, dtype=np.int64)
    np.cumsum(counts, out=starts[1:])

    # global pairing: sort experts by count desc, core c gets ranks (c, 15-c);
    # slot 0 holds the larger one. Minimizes both slot capacities:
    # caps = (count of rank 0, count of rank NCORES) rounded up to 128.
    def r128(v):
        return max(P, int(-(-v // P)) * P)

    by_size = sorted(range(E), key=lambda e: -counts[e])
    slot_experts = [
        [by_size[c], by_size[E - 1 - c]] for c in range(NCORES)
    ]  # [core][slot] -> expert id
    caps = tuple(
        int(max(counts[slot_experts[c][s]] for c in range(NCORES)))
        for s in range(EPC)
    )
    lcaps = [r128(Cs) for Cs in caps]
    CTOT = sum(lcaps)
    slot_chunks = [
        _chunks_of(Cs, lead=256 if s == 0 else 0) for s, Cs in enumerate(caps)
    ]
    slot_off = np.cumsum([0] + list(lcaps))
    # flat xt layout: chunks packed back to back, each [P, DT, tch] row-major
    xt_offs, xoff = [], 0
    for chs in slot_chunks:
        offs = []
        for (_, tch) in chs:
            offs.append(xoff)
            xoff += P * DT * tch
        xt_offs.append(offs)
    XTELEMS = xoff

    xt_all = np.zeros((NCORES, XTELEMS), dtype=BF16)
    cw_all = np.zeros((NCORES, P, CTOT // P), dtype=np.float32)
    wg_all = np.zeros((NCORES, EPC, FT, P, DT, P), dtype=BF16)
    wu_all = np.zeros((NCORES, EPC, FT, P, DT, P), dtype=BF16)
    wd_all = np.zeros((NCORES, EPC, P, FT, D), dtype=BF16)

    tok_lists = {}
    for c in range(NCORES):
        for s in range(EPC):
            e = slot_experts[c][s]
            toks = gt[starts[e] : starts[e + 1]]
            tok_lists[(c, s)] = toks
            ne = len(toks)
            # tokens, transposed + tiled per chunk: [p, d, c] = X[tok, d*128+p]
            for ci, (t0, tch) in enumerate(slot_chunks[s]):
                sel = toks[t0 : min(t0 + tch, ne)]
                blk = np.zeros((P, DT, tch), dtype=BF16)
                if len(sel):
                    blk[:, :, : len(sel)] = (
                        xf[sel]
                        .astype(BF16)
                        .reshape(len(sel), DT, P)
                        .transpose(2, 1, 0)
                    )
                xo = xt_offs[s][ci]
                xt_all[c, xo : xo + P * DT * tch] = blk.ravel()
            cw_flat = np.zeros(lcaps[s], dtype=np.float32)
            cw_flat[:ne] = gw[starts[e] : starts[e + 1]]
            cw_all[c, :, slot_off[s] // P : slot_off[s + 1] // P] = (
                cw_flat.reshape(-1, P).T
            )
            # weights, partition-major K tiles
            wg_all[c, s] = (
                Wg[e].astype(BF16).reshape(DT, P, FT, P).transpose(2, 1, 0, 3)
            )
            wu_all[c, s] = (
                Wu[e].astype(BF16).reshape(DT, P, FT, P).transpose(2, 1, 0, 3)
            )
            wd_all[c, s] = Wd[e].astype(BF16).reshape(FT, P, D).transpose(1, 0, 2)

    nc = _get_program(caps)
    in_maps = [
        {
            "xt": xt_all[c],
            "wg": wg_all[c],
            "wu": wu_all[c],
            "wd": wd_all[c],
            "cw": cw_all[c],
        }
        for c in range(NCORES)
    ]
    kwargs = {}
    if trace:
        _ensure_ntff_hook()
        kwargs = dict(trace=True, trace_cores=list(range(NCORES)))
    res = bass_utils.run_bass_kernel_spmd(
        nc, in_maps, core_ids=list(range(NCORES)), **kwargs
    )

    out = np.zeros((N, D), dtype=np.float32)
    for c in range(NCORES):
        yc = res.results[c]["y"].reshape(CTOT, D)
        for s in range(EPC):
            toks = tok_lists[(c, s)]
            out[toks] += yc[slot_off[s] : slot_off[s] + len(toks)].astype(
                np.float32
            )
    return out.reshape(B, S, D), res.exec_time_ns


# Pre-register the NTFF hook shim at import: if the grading harness sets
# BASS_TRACE=1, run_bass_kernel_spmd's axon trace path imports
# antenv.axon_hooks, which the container's antenv stub lacks.
try:
    _ensure_ntff_hook()
except Exception:
    pass


def kernel(**inputs) -> np.ndarray:
    out, _ = _run(inputs, trace=False)
    return out
